# revision 1
# baseline (speedup 1.0000x reference)
"""Trainium2 Bass kernel for nn_LSMTradingModel_49168785605378.

Dataflow analysis of the reference:
  lif_step(inp, v, i) returns (z, v_new, i_new) where z and v_new depend
  only on (v, i) -- `inp` feeds i_new exclusively.  The reference keeps
  only z3 and v3n from the third LIF layer and discards every i_new, so
  the whole output is a pure elementwise function of v3 and i3:

      c     = f32(1e-3 * (1/3))            # DT * tau_mem_inv
      v_dec = v3 + c * ((0 - v3) + i3)
      z3    = (v_dec - 0.1 > 0) ? 1.0 : 0.0
      v3n   = (1 - z3) * v_dec             # == (v_dec <= 0.1) * v_dec

  x, w_in, w_out, v1, i1, v2, i2 are dead inputs (verified bit-exact
  against the jax reference under random perturbation of those inputs).

Sharding: pure data parallel, batch B=131072 split across 8 cores.
Each core handles 16384 rows x 2 cols = 32768 f32 per tensor, viewed
on-chip as [128 partitions x 256 cols].  v3/i3 are packed host-side
into one DRAM input per core (column-chunk interleaved so each chunk's
load is a single contiguous-row DMA), z3/v3n likewise share one DRAM
output per core.
"""

from contextlib import ExitStack

import numpy as np

N_CORES = 8
B = 131072
SH = B // N_CORES  # rows per core: 16384
P = 128  # SBUF partitions
F = SH * 2 // P  # flattened free-dim cols per tensor per core: 256
NCH = 2  # column chunks per core (DMA/compute pipelining)
CC = F // NCH  # v3-cols per chunk
C_DECAY = float(np.float32(1e-3 * (1.0 / 3.0)))  # DT * tau_mem_inv, f32-exact

_cache: dict = {}


def _build_nc():
    import concourse.tile as tile
    from concourse import bacc, mybir

    f32 = mybir.dt.float32
    op = mybir.AluOpType

    nc = bacc.Bacc(
        "TRN2",
        target_bir_lowering=False,
        debug=False,
        enable_asserts=False,
        num_devices=1,
    )
    vi = nc.dram_tensor("vi", [P, 2 * F], f32, kind="ExternalInput").ap()
    zo = nc.dram_tensor("zo", [P, 2 * F], f32, kind="ExternalOutput").ap()

    with tile.TileContext(nc) as tc, ExitStack() as ctx:
        io_pool = ctx.enter_context(tc.tile_pool(name="io", bufs=NCH))
        tmp_pool = ctx.enter_context(tc.tile_pool(name="tmp", bufs=NCH))
        for g in range(NCH):
            lo, hi = 2 * CC * g, 2 * CC * (g + 1)
            tin = io_pool.tile([P, 2 * CC], f32, tag="tin")
            nc.sync.dma_start(tin[:], vi[:, lo:hi])
            v3 = tin[:, 0:CC]
            i3 = tin[:, CC : 2 * CC]

            t = tmp_pool.tile([P, CC], f32, tag="t")
            vdec = tmp_pool.tile([P, CC], f32, tag="vdec")
            tout = io_pool.tile([P, 2 * CC], f32, tag="tout")

            # t = (0 - v3) + i3  (== i3 - v3 bit-exactly)
            nc.vector.tensor_tensor(t[:], i3, v3, op.subtract)
            # v_dec = (t * c) + v3
            nc.vector.scalar_tensor_tensor(
                vdec[:], t[:], C_DECAY, v3, op.mult, op.add
            )
            # z3 = ((v_dec - 0.1) > 0) -> 1.0/0.0
            nc.vector.tensor_scalar(
                tout[:, 0:CC], vdec[:], 0.1, 0.0, op.subtract, op.is_gt
            )
            # v3n = (v_dec <= 0.1) * v_dec   (== (1 - z3) * v_dec)
            nc.vector.scalar_tensor_tensor(
                tout[:, CC : 2 * CC], vdec[:], 0.1, vdec[:], op.is_le, op.mult
            )
            nc.sync.dma_start(zo[:, lo:hi], tout[:])
    nc.compile()
    return nc


def _get_nc():
    if "nc" not in _cache:
        _cache["nc"] = _build_nc()
    return _cache["nc"]


def _pack_in_maps(v3, i3):
    v3 = np.ascontiguousarray(np.asarray(v3, dtype=np.float32))
    i3 = np.ascontiguousarray(np.asarray(i3, dtype=np.float32))
    in_maps = []
    for c in range(N_CORES):
        v = v3[c * SH : (c + 1) * SH].reshape(P, F)
        i = i3[c * SH : (c + 1) * SH].reshape(P, F)
        buf = np.empty((P, 2 * F), np.float32)
        for g in range(NCH):
            buf[:, 2 * CC * g : 2 * CC * g + CC] = v[:, g * CC : (g + 1) * CC]
            buf[:, 2 * CC * g + CC : 2 * CC * (g + 1)] = i[:, g * CC : (g + 1) * CC]
        in_maps.append({"vi": buf})
    return in_maps


def _unpack_results(results):
    z3 = np.empty((B, 2), np.float32)
    v3n = np.empty((B, 2), np.float32)
    zc = np.empty((P, F), np.float32)
    vc = np.empty((P, F), np.float32)
    for c in range(N_CORES):
        out = results[c]["zo"]
        for g in range(NCH):
            zc[:, g * CC : (g + 1) * CC] = out[:, 2 * CC * g : 2 * CC * g + CC]
            vc[:, g * CC : (g + 1) * CC] = out[:, 2 * CC * g + CC : 2 * CC * (g + 1)]
        z3[c * SH : (c + 1) * SH] = zc.reshape(SH, 2)
        v3n[c * SH : (c + 1) * SH] = vc.reshape(SH, 2)
    return z3, v3n


def run(inputs: dict, trace: bool = False):
    """Run on 8 NeuronCores. Returns ((z3, v3n), BassKernelResults)."""
    from concourse.bass_utils import run_bass_kernel_spmd

    nc = _get_nc()
    in_maps = _pack_in_maps(inputs["v3"], inputs["i3"])
    res = run_bass_kernel_spmd(nc, in_maps, list(range(N_CORES)), trace=trace)
    return _unpack_results(res.results), res


def kernel(x, w_in, w_out, v1, i1, v2, i2, v3, i3):
    (z3, v3n), _ = run({"v3": v3, "i3": i3})
    return z3, v3n
